# revision 1
# baseline (speedup 1.0000x reference)
"""Trainium2 Bass kernel for nn_DualContrastiveModel (GAT-style relational attention).

Math per batch b (N=256 nodes, D=128 features, 4 relation types):
    g_r[i,j] = sum_d h[i,d]*a_r[d]*h[j,d]          (4 symmetric bilinear score matrices)
    scores   = g_{adj-1} where adj in {1..4}, -inf where adj==0
    alpha    = softmax(leakyrelu(scores), axis=-1)  (slope 0.2)
    out      = alpha @ h

Kernel strategy (8 cores, data-parallel over batch; v2, ~2.1-2.8 us/batch/core
vs ~3.4-4.1 for the v1 baseline on the same box):
  - scores are computed directly in transposed (j-major) layout so no PE
    transposes and no separate mask-inject matmuls are needed:
      t_r[j,i] = sum_d hT[d,j]*hw_r[d,i] + 192*mask_r[j,i]
    via fp8 DoubleRow matmuls: DR gives a virtual K=256 contraction =
    [d-contraction ; j'-identity], the mask riding the upper half against a
    +-192*I stationary.  All h-derived operands (hT, hw := a_r (.) hT, x|1)
    are pre-cast/packed on the host, which removes the per-batch transposes,
    PSUM->SBUF rounding copies and gpsimd scaling passes of v1 entirely.
  - fp8_e4m3 precision is recovered with an error-feedback split (hi/lo fp8
    pairs, lo = fp8(x - hi)): MM-A = [hThi; hThi] x (hwhi, hwlo) and
    MM-B = [hTlo; +-192I] x (hwhi, mask) accumulate all cross terms except
    lo*lo in PSUM.  The mask's DR cell-pair partner must be the negligible
    hTlo product: the DR pair-sum rounds internally, and pairing the +-192
    bias with a main-score product swallows it (measured 2.1e-2 rel err vs
    6.3e-3 with this pairing; plain fp8 without the split also fails at
    2.2e-2.  bf16 v1 reference is ~3e-4; gate is 2e-2).
  - relation selection: host ships S = [adj==1]-[adj==4], T = [adj==2]-[adj==3]
    transposed as fp8 {-1,0,1}; bias b = (+192S, +192T, -192T, -192S) makes
    t_sel = g_sel + 192 dominate.  PSUM plane order (t0,t1,t3,t2) lets both
    relation pairs stream the same [S|T] moving AP (only the +-192I stationary
    half flips sign).  The 4-way max is ONE strided DVE tensor_reduce per
    j-half from PSUM (DVE has a single PSUM read port - tensor_tensor with two
    PSUM operands is rejected by the walrus verifier, and gpsimd has no max -
    so the reduce is the minimal-read select).  ACT Prelu(bias=-192) eats the
    offset for free, then Exp -> f16.
  - output matmul: po[i,(d|s)] = sum_j pT[j,i]*[h|1][j,:] with a ones column
    for the softmax row-sums.  Both per-half 1/s row scales run on DVE
    (tensor_tensor mult with a broadcast [P,1] scalar): keeping them on ACT
    serializes its Prelu/Exp queue and costs >1.3 us/batch (measured;
    moving them off ACT took a 64-batch pass from ~221 us to ~183 us).
  - emission is software-pipelined 5 deep (head / scores+select / prelu+exp /
    out-matmul / normalize+store) so the Tile scheduler overlaps batches
    across engines.  PSUM: 3x2 banks score tiles + 2x1 bank output tiles.
"""

import os
import sys

import numpy as np

for _p in ("/root/.axon_site/_ro/trn_rl_repo", "/opt/trn_rl_repo"):
    if os.path.isdir(_p) and _p not in sys.path:
        sys.path.append(_p)

_BASS_STATE = {}

BIG = 192.0


def _build_program(Bshard: int, repeat: int = 1):
    from contextlib import ExitStack, nullcontext

    import concourse.bacc as bacc
    import concourse.mybir as mybir
    import concourse.tile as tile
    from concourse.masks import make_identity

    f32 = mybir.dt.float32
    f16 = mybir.dt.float16
    f8 = mybir.dt.float8e4
    N, D = 256, 128
    P = 128

    nc = bacc.Bacc(
        "TRN2",
        target_bir_lowering=False,
        debug=False,
        enable_asserts=False,
        num_devices=8,
    )
    # wht planes: {0: hTlo_J0, 1: hTlo_J1, 2: hThi_J0, 3: hThi_J1,
    #              4: hThi_J0 (dup), 5: hThi_J1 (dup), 6: +192I, 7: -192I}
    wht_d = nc.dram_tensor("wht", [Bshard, P, 8, 128], f8, kind="ExternalInput").ap()
    # mv planes (512 wide): {0: hwhi[r0|r1], 1: hwhi[r3|r2], 2: hwlo[r0|r1],
    #                        3: hwlo[r3|r2], 4: [S|T] rows j=0..127, 5: [S|T] rows 128..255}
    mv_d = nc.dram_tensor("mv", [Bshard, P, 6, 512], f8, kind="ExternalInput").ap()
    # xt: [h | 1] rows, f16
    xt_d = nc.dram_tensor("xt", [Bshard, P, 2, 129], f16, kind="ExternalInput").ap()
    # out[b, p, I, d] = result[b, I*128+p, d], f16
    out_d = nc.dram_tensor("out", [Bshard, P, 2, 128], f16, kind="ExternalOutput").ap()

    with tile.TileContext(nc) as tc:
        with ExitStack() as ctx:
            ep = ctx.enter_context

            consts = ep(tc.tile_pool(name="consts", bufs=1))
            negbig = consts.tile([P, 1], f32)
            nc.vector.memset(negbig, -BIG)

            wht_p = ep(tc.tile_pool(name="wht", bufs=5))
            mv_p = ep(tc.tile_pool(name="mv", bufs=5))
            xt_p = ep(tc.tile_pool(name="xt", bufs=7))
            sel_p = ep(tc.tile_pool(name="sel", bufs=4))
            pl_p = ep(tc.tile_pool(name="pl", bufs=4))
            pT_p = ep(tc.tile_pool(name="pT", bufs=5))
            rs_p = ep(tc.tile_pool(name="rs", bufs=4))
            ob_p = ep(tc.tile_pool(name="ob", bufs=4))

            tps_p = ep(tc.tile_pool(name="tps", bufs=3, space="PSUM"))
            pos_p = ep(tc.tile_pool(name="pos", bufs=2, space="PSUM"))

            AX = mybir.AxisListType.X
            OP = mybir.AluOpType
            AF = mybir.ActivationFunctionType
            DR = mybir.MatmulPerfMode.DoubleRow

            def emit_head(b):
                st = {}
                wht = wht_p.tile([P, 8, 128], f8, tag="wht", name=f"wht{b}")
                nc.sync.dma_start(wht, wht_d[b])
                mv = mv_p.tile([P, 6, 512], f8, tag="mv", name=f"mv{b}")
                nc.sync.dma_start(mv, mv_d[b])
                xt = xt_p.tile([P, 2, 129], f16, tag="xt", name=f"xt{b}")
                nc.sync.dma_start(xt, xt_d[b])
                st["wht"], st["mv"], st["xt"] = wht, mv, xt
                return st

            def emit_score(b, st):
                wht, mv = st["wht"], st["mv"]
                sel = sel_p.tile([P, 2, N], f32, tag="sel", name=f"sel{b}")
                st["sel"] = sel
                for J in range(2):
                    tp = tps_p.tile([P, 4, N], f32, tag="tps", name=f"tp{b}_{J}")
                    lA = wht[:, 2 + J : 5 + J : 2, :]  # [hThi_J; hThi_J dup]
                    # both MM-A first (shared stationary), then both MM-B:
                    # consecutive same-lhsT matmuls avoid LDWEIGHTS serialization
                    for q in range(2):
                        # bank q holds planes (t0,t1) for q=0, (t3,t2) for q=1
                        # MM-A: hThi.hwhi + hThi.hwlo
                        nc.tensor.matmul(
                            tp[:, 2 * q : 2 * q + 2, :],
                            lhsT=lA,
                            rhs=mv[:, q : q + 3 : 2, :],  # (hwhi_q, hwlo_q)
                            start=True,
                            stop=False,
                            perf_mode=DR,
                        )
                    for q in range(2):
                        # MM-B: hTlo.hwhi + (+-192)*mask -- the mask's DR cell-pair
                        # partner is the negligible hTlo product, so the DR
                        # pair-sum rounding cannot swallow a main-score term
                        iw = 6 + q  # +192I for q=0, -192I for q=1
                        lB = wht[:, J : iw + 1 : iw - J, :]
                        nc.tensor.matmul(
                            tp[:, 2 * q : 2 * q + 2, :],
                            lhsT=lB,
                            rhs=mv[:, q : 4 + J + 1 : 4 + J - q, :],  # (hwhi_q, masks_J)
                            start=False,
                            stop=True,
                            perf_mode=DR,
                        )
                    # 4-way relation select: strided max-reduce from PSUM
                    nc.vector.tensor_reduce(
                        sel[:, J, :], tp.rearrange("p r i -> p i r"),
                        axis=AX, op=OP.max,
                    )

            def emit_sel(b, st):
                sel = st["sel"]
                # prelu(sel - 192) then exp; ACT affine eats the offset
                pl = pl_p.tile([P, 2, N], f16, tag="pl", name=f"pl{b}")
                nc.scalar.activation(pl, sel, AF.Prelu, bias=negbig, alpha=0.2)
                pT = pT_p.tile([P, 2, N], f16, tag="pT", name=f"pT{b}")
                nc.scalar.activation(pT, pl, AF.Exp)
                st["pT"] = pT

            def emit_out(b, st):
                pT, xt = st["pT"], st["xt"]
                po = pos_p.tile([P, 2, D + 1], f32, tag="pos", name=f"po{b}")
                st["po"] = po
                for I in range(2):
                    for J in range(2):
                        nc.tensor.matmul(
                            po[:, I, :],
                            lhsT=pT[:, J, I * P : (I + 1) * P],
                            rhs=xt[:, J, :],
                            start=(J == 0),
                            stop=(J == 1),
                        )

            def emit_fin(b, st):
                po = st["po"]
                rs = rs_p.tile([P, 2], f32, tag="rs", name=f"rs{b}")
                nc.vector.reciprocal(rs, po[:, :, D])
                ob = ob_p.tile([P, 2, D], f16, tag="ob", name=f"ob{b}")
                # row scales: both on DVE (TT-mult, PSUM + broadcast-SBUF scalar);
                # ACT keeps only Prelu+Exp -- extra ACT queue entries serialize it
                for I in range(2):
                    nc.vector.tensor_tensor(
                        ob[:, I, :], po[:, I, 0:D], rs[:, I : I + 1].broadcast_to([P, D]),
                        op=OP.mult,
                    )
                nc.sync.dma_start(out_d[b], ob)

            loop_cm = tc.For_i(0, repeat, 1) if repeat > 1 else nullcontext()
            with loop_cm:
                sts = {}
                for b in range(Bshard + 4):
                    if b < Bshard:
                        sts[b] = emit_head(b)
                    if 1 <= b <= Bshard:
                        emit_score(b - 1, sts[b - 1])
                    if 2 <= b <= Bshard + 1:
                        emit_sel(b - 2, sts[b - 2])
                    if 3 <= b <= Bshard + 2:
                        emit_out(b - 3, sts[b - 3])
                    if b >= 4:
                        emit_fin(b - 4, sts.pop(b - 4))

    nc.compile()
    return nc


def _get_program(Bshard: int):
    key = ("prog", Bshard)
    if key not in _BASS_STATE:
        _BASS_STATE[key] = _build_program(Bshard)
    return _BASS_STATE[key]


def pack_inputs(hidden: np.ndarray, adj: np.ndarray, apack: np.ndarray):
    """Host-side packing of full inputs into the kernel's DRAM tensors.

    hidden: [B, N, D] f32; adj: [B, N, N] int; apack: [4, D] f32.
    Returns dict of full (unsharded) arrays: wht, mv, xt.
    """
    import ml_dtypes

    F8 = ml_dtypes.float8_e4m3
    B, N, D = hidden.shape
    P = 128
    f32 = np.float32

    hT = np.ascontiguousarray(hidden.transpose(0, 2, 1)).astype(f32)  # [B, D, N]
    hThi = hT.astype(F8)
    hTlo = (hT - hThi.astype(f32)).astype(F8)

    wht = np.zeros((B, P, 8, 128), dtype=F8)
    wht[:, :, 0, :] = hTlo[:, :, 0:128]
    wht[:, :, 1, :] = hTlo[:, :, 128:256]
    wht[:, :, 2, :] = hThi[:, :, 0:128]
    wht[:, :, 3, :] = hThi[:, :, 128:256]
    wht[:, :, 4, :] = wht[:, :, 2, :]
    wht[:, :, 5, :] = wht[:, :, 3, :]
    ident = np.eye(128, dtype=f32)
    wht[:, :, 6, :] = (BIG * ident).astype(F8)[None]
    wht[:, :, 7, :] = (-BIG * ident).astype(F8)[None]

    # hw_r = a_r (.) hT, hi/lo fp8 split, relation plane order [r0, r1, r3, r2]
    hwf = apack[None, :, :, None] * hT[:, None, :, :]  # [B, 4, D, N]
    hwhi = hwf.astype(F8)
    hwlo = (hwf - hwhi.astype(f32)).astype(F8)
    mv = np.zeros((B, P, 6, 512), dtype=F8)
    mv[:, :, 0, 0:256] = hwhi[:, 0]
    mv[:, :, 0, 256:512] = hwhi[:, 1]
    mv[:, :, 1, 0:256] = hwhi[:, 3]
    mv[:, :, 1, 256:512] = hwhi[:, 2]
    mv[:, :, 2, 0:256] = hwlo[:, 0]
    mv[:, :, 2, 256:512] = hwlo[:, 1]
    mv[:, :, 3, 0:256] = hwlo[:, 3]
    mv[:, :, 3, 256:512] = hwlo[:, 2]
    # masks S = [adj==1]-[adj==4], T = [adj==2]-[adj==3], transposed to [j, i]
    adjT = adj.transpose(0, 2, 1)
    S = ((adjT == 1).astype(f32) - (adjT == 4)).astype(F8)  # [B, j, i]
    T = ((adjT == 2).astype(f32) - (adjT == 3)).astype(F8)
    mv[:, :, 4, 0:256] = S[:, 0:128, :]
    mv[:, :, 4, 256:512] = T[:, 0:128, :]
    mv[:, :, 5, 0:256] = S[:, 128:256, :]
    mv[:, :, 5, 256:512] = T[:, 128:256, :]

    xt = np.ones((B, P, 2, 129), dtype=np.float16)
    h4 = hidden.reshape(B, 2, 128, D)  # [B, I, p, D]
    xt[:, :, :, 0:128] = h4.transpose(0, 2, 1, 3).astype(np.float16)

    return {"wht": wht, "mv": mv, "xt": xt}


def unpack_output(out: np.ndarray) -> np.ndarray:
    """[B, p, I, d] f16 -> [B, N, D] f32."""
    B = out.shape[0]
    return (
        out.transpose(0, 2, 1, 3).reshape(B, 256, 128).astype(np.float32)
    )


def kernel(hidden: np.ndarray, adj: np.ndarray, a_0, a_1, a_2, a_3) -> np.ndarray:
    from concourse import bass_utils

    B, N, D = hidden.shape
    NCORES = 8
    assert B % NCORES == 0
    Bs = B // NCORES

    apack = np.ascontiguousarray(
        np.concatenate([a_0, a_1, a_2, a_3], axis=1).T.astype(np.float32)
    )  # [4, D]
    hidden = np.ascontiguousarray(hidden, dtype=np.float32)
    packed = pack_inputs(hidden, np.asarray(adj), apack)

    nc = _get_program(Bs)
    in_maps = [
        {k: v[c * Bs : (c + 1) * Bs] for k, v in packed.items()} for c in range(NCORES)
    ]
    res = bass_utils.run_bass_kernel_spmd(
        nc,
        in_maps,
        core_ids=list(range(NCORES)),
        trace=bool(int(os.environ.get("KERNEL_TRACE", "0"))),
    )
    _BASS_STATE["last_result"] = res
    return unpack_output(np.concatenate([r["out"] for r in res.results], axis=0))



# revision 4
# speedup vs baseline: 1.1564x; 1.1564x over previous
"""Trainium2 Bass kernel for nn_DualContrastiveModel (GAT-style relational attention).

Math per batch b (N=256 nodes, D=128 features, 4 relation types):
    g_r[i,j] = sum_d h[i,d]*a_r[d]*h[j,d]          (4 symmetric bilinear score matrices)
    scores   = g_{adj-1} where adj in {1..4}, -inf where adj==0
    alpha    = softmax(leakyrelu(scores), axis=-1)  (slope 0.2)
    out      = alpha @ h

Kernel strategy (8 cores, data-parallel over batch).  v3 changes vs v2:
  - DMA is chunked: G=4 batches ride in ONE f8 load (wht planes ++ mv planes,
    4096 B/partition/batch) plus one f16 load (xt) and one f16 store per
    chunk.  v2 issued 4 dma_starts per batch; at ~565 ns SP-sequencer +
    ~625 ns HWDGE per descriptor-gen, DMA *issue* (not bandwidth) was a
    near-critical serial path in the timeline model.
  - the 4-way relation select is split across engines per j-half: J0 keeps
    the v2 single strided DVE max-reduce from PSUM (1024 f32 elems, the
    minimal-read DVE select) while J1 crosses PSUM->SBUF on ACT as
    Prelu(x-192, 0.2) over all 4 planes (prelu commutes with max) and then
    takes a cheap f16 TT-max tree on DVE (f16 contiguous SBUF qualifies for
    the DVE 2x mode; TensorReduce never does).  This moves ~840 ns/batch of
    the old 2x1192 ns DVE crossing onto the ACT engine, balancing
    DVE ~2.36 us / ACT ~2.19 us per batch in the cost model vs 3.03/1.38 in
    v2 (Pool/gpsimd cannot touch PSUM and rejects TensorTensor entirely, so
    it cannot help).
  - fp8 DoubleRow score matmuls with hi/lo error-feedback split and the
    +-192*mask DR bias injection are unchanged from v2 (see below).

v2 notes that still apply:
  - scores are computed directly in transposed (j-major) layout:
      t_r[j,i] = sum_d hT[d,j]*hw_r[d,i] + 192*mask_r[j,i]
    via fp8 DoubleRow matmuls: MM-A = [hThi; hThi] x (hwhi, hwlo) and
    MM-B = [hTlo; +-192I] x (hwhi, mask) accumulate all cross terms except
    lo*lo in PSUM (measured 6.3e-3 rel err; plain fp8 fails at 2.2e-2).
  - relation selection: host ships S = [adj==1]-[adj==4], T = [adj==2]-[adj==3]
    transposed as fp8 {-1,0,1}; bias b = (+192S, +192T, -192T, -192S) with
    PSUM plane order (t0,t1,t3,t2) makes t_sel = g_sel + 192 dominate.
    ACT Prelu(bias=-192) eats the offset, then Exp -> f16.
  - output matmul: po[i,(d|s)] = sum_j pT[j,i]*[h|1][j,:] with a ones column
    for the softmax row-sums; per-half 1/s row scales on DVE.
"""

import os
import sys

import numpy as np

for _p in ("/root/.axon_site/_ro/trn_rl_repo", "/opt/trn_rl_repo"):
    if os.path.isdir(_p) and _p not in sys.path:
        sys.path.append(_p)

_BASS_STATE = {}

BIG = 192.0
G = 4  # batches per DMA chunk


def _build_program(Bshard: int, repeat: int = 1):
    from contextlib import ExitStack, nullcontext

    import concourse.bacc as bacc
    import concourse.mybir as mybir
    import concourse.tile as tile

    f32 = mybir.dt.float32
    f16 = mybir.dt.float16
    f8 = mybir.dt.float8e4
    N, D = 256, 128
    P = 128
    assert Bshard % G == 0
    NC = Bshard // G

    nc = bacc.Bacc(
        "TRN2",
        target_bir_lowering=False,
        debug=False,
        enable_asserts=False,
        num_devices=8,
    )
    # f8 chunk: per batch 4096 B/partition = wht 8 planes x 128 ++ mv 6 x 512
    # wht planes: {0: hTlo_J0, 1: hTlo_J1, 2: hThi_J0, 3: hThi_J1,
    #              4: hThi_J0 (dup), 5: hThi_J1 (dup), 6: +192I, 7: -192I}
    # mv planes (512 wide): {0: hwhi[r0|r1], 1: hwhi[r3|r2], 2: hwlo[r0|r1],
    #                        3: hwlo[r3|r2], 4: [S|T] rows j=0..127, 5: rows 128..255}
    in8_d = nc.dram_tensor("in8", [NC, P, G, 4096], f8, kind="ExternalInput").ap()
    # f16 chunk: xt = [h | 1] rows, 258 f16/partition/batch
    in16_d = nc.dram_tensor("in16", [NC, P, G, 258], f16, kind="ExternalInput").ap()
    # out[c, p, g, I, d] = result[(c*G+g), I*128+p, d], f16
    out_d = nc.dram_tensor("out", [NC, P, G, 2, 128], f16, kind="ExternalOutput").ap()

    with tile.TileContext(nc) as tc:
        with ExitStack() as ctx:
            ep = ctx.enter_context

            consts = ep(tc.tile_pool(name="consts", bufs=1))
            negbig = consts.tile([P, 1], f32)
            nc.vector.memset(negbig, -BIG)

            c8_p = ep(tc.tile_pool(name="c8", bufs=3))
            c16_p = ep(tc.tile_pool(name="c16", bufs=3))
            ob_p = ep(tc.tile_pool(name="ob", bufs=2))

            sel_p = ep(tc.tile_pool(name="sel", bufs=4))
            cr_p = ep(tc.tile_pool(name="cr", bufs=4))
            m2_p = ep(tc.tile_pool(name="m2", bufs=4))
            m1_p = ep(tc.tile_pool(name="m1", bufs=4))
            pl_p = ep(tc.tile_pool(name="pl", bufs=4))
            pT0_p = ep(tc.tile_pool(name="pT0", bufs=5))
            pT1_p = ep(tc.tile_pool(name="pT1", bufs=5))
            rs_p = ep(tc.tile_pool(name="rs", bufs=4))

            tps_p = ep(tc.tile_pool(name="tps", bufs=3, space="PSUM"))
            pos_p = ep(tc.tile_pool(name="pos", bufs=2, space="PSUM"))

            AX = mybir.AxisListType.X
            OP = mybir.AluOpType
            AF = mybir.ActivationFunctionType
            DR = mybir.MatmulPerfMode.DoubleRow

            chunks = {}

            def load_chunk(c):
                t8 = c8_p.tile([P, G, 4096], f8, tag="c8", name=f"c8_{c}")
                nc.sync.dma_start(t8, in8_d[c])
                t16 = c16_p.tile([P, G, 258], f16, tag="c16", name=f"c16_{c}")
                nc.sync.dma_start(t16, in16_d[c])
                chunks[c] = (t8, t16)

            def emit_head(b):
                # prefetch the next chunk at each chunk boundary
                if b % G == 0:
                    c = b // G + 1
                    if c < NC:
                        load_chunk(c)
                st = {}
                t8, t16 = chunks[b // G]
                g = b % G
                st["wht"] = t8[:, g, 0:1024].rearrange("p (r c) -> p r c", r=8)
                st["mv"] = t8[:, g, 1024:4096].rearrange("p (r c) -> p r c", r=6)
                st["xt"] = t16[:, g, :].rearrange("p (r c) -> p r c", r=2)
                return st

            def emit_score(b, st):
                wht, mv = st["wht"], st["mv"]
                for J in range(2):
                    tp = tps_p.tile([P, 4, N], f32, tag="tps", name=f"tp{b}_{J}")
                    lA = wht[:, 2 + J : 5 + J : 2, :]  # [hThi_J; hThi_J dup]
                    # both MM-A first (shared stationary), then both MM-B:
                    # consecutive same-lhsT matmuls avoid LDWEIGHTS serialization
                    for q in range(2):
                        # bank q holds planes (t0,t1) for q=0, (t3,t2) for q=1
                        # MM-A: hThi.hwhi + hThi.hwlo
                        nc.tensor.matmul(
                            tp[:, 2 * q : 2 * q + 2, :],
                            lhsT=lA,
                            rhs=mv[:, q : q + 3 : 2, :],  # (hwhi_q, hwlo_q)
                            start=True,
                            stop=False,
                            perf_mode=DR,
                        )
                    for q in range(2):
                        # MM-B: hTlo.hwhi + (+-192)*mask -- the mask's DR cell-pair
                        # partner is the negligible hTlo product, so the DR
                        # pair-sum rounding cannot swallow a main-score term
                        iw = 6 + q  # +192I for q=0, -192I for q=1
                        lB = wht[:, J : iw + 1 : iw - J, :]
                        nc.tensor.matmul(
                            tp[:, 2 * q : 2 * q + 2, :],
                            lhsT=lB,
                            rhs=mv[:, q : 4 + J + 1 : 4 + J - q, :],  # (hwhi_q, masks_J)
                            start=False,
                            stop=True,
                            perf_mode=DR,
                        )
                    if J == 0:
                        # J0 select: single strided max-reduce from PSUM on DVE
                        sel = sel_p.tile([P, N], f32, tag="sel", name=f"sel{b}")
                        nc.vector.tensor_reduce(
                            sel, tp.rearrange("p r i -> p i r"), axis=AX, op=OP.max
                        )
                        st["sel0"] = sel
                    else:
                        # J1 crossing on ACT: prelu(t - 192) over all 4 planes
                        # (prelu is monotone, so it commutes with the max)
                        cr = cr_p.tile([P, 4, N], f16, tag="cr", name=f"cr{b}")
                        nc.scalar.activation(cr, tp, AF.Prelu, bias=negbig, alpha=0.2)
                        st["cr1"] = cr

            def emit_sel(b, st):
                # J0: prelu(sel - 192) then exp on ACT
                pl = pl_p.tile([P, N], f16, tag="pl", name=f"pl{b}")
                nc.scalar.activation(pl, st["sel0"], AF.Prelu, bias=negbig, alpha=0.2)
                pT0 = pT0_p.tile([P, N], f16, tag="pT0", name=f"pT0_{b}")
                nc.scalar.activation(pT0, pl, AF.Exp)
                st["pT0"] = pT0
                # J1: f16 TT-max tree on DVE (2x mode), then exp on ACT
                cr = st["cr1"]
                m2 = m2_p.tile([P, 2, N], f16, tag="m2", name=f"m2_{b}")
                nc.vector.tensor_tensor(m2, cr[:, 0:2, :], cr[:, 2:4, :], op=OP.max)
                m1 = m1_p.tile([P, N], f16, tag="m1", name=f"m1_{b}")
                nc.vector.tensor_tensor(m1, m2[:, 0, :], m2[:, 1, :], op=OP.max)
                pT1 = pT1_p.tile([P, N], f16, tag="pT1", name=f"pT1_{b}")
                nc.scalar.activation(pT1, m1, AF.Exp)
                st["pT1"] = pT1

            def emit_out(b, st):
                xt = st["xt"]
                po = pos_p.tile([P, 2, D + 1], f32, tag="pos", name=f"po{b}")
                st["po"] = po
                for I in range(2):
                    for J in range(2):
                        pT = st["pT0"] if J == 0 else st["pT1"]
                        nc.tensor.matmul(
                            po[:, I, :],
                            lhsT=pT[:, I * P : (I + 1) * P],
                            rhs=xt[:, J, :],
                            start=(J == 0),
                            stop=(J == 1),
                        )

            def emit_fin(b, st):
                po = st["po"]
                c = b // G
                g = b % G
                if g == 0:
                    st["obt"] = ob_p.tile([P, G, 2, D], f16, tag="ob", name=f"ob{c}")
                    chunks[("ob", c)] = st["obt"]
                obt = chunks[("ob", c)]
                rs = rs_p.tile([P, 2], f32, tag="rs", name=f"rs{b}")
                nc.vector.reciprocal(rs, po[:, :, D])
                # row scales: both on DVE (TT-mult, PSUM + broadcast-SBUF scalar)
                for I in range(2):
                    nc.vector.tensor_tensor(
                        obt[:, g, I, :],
                        po[:, I, 0:D],
                        rs[:, I : I + 1].broadcast_to([P, D]),
                        op=OP.mult,
                    )
                if g == G - 1:
                    nc.sync.dma_start(out_d[c], obt)
                    del chunks[("ob", c)]

            loop_cm = tc.For_i(0, repeat, 1) if repeat > 1 else nullcontext()
            with loop_cm:
                chunks.clear()
                load_chunk(0)
                sts = {}
                for b in range(Bshard + 4):
                    if b < Bshard:
                        sts[b] = emit_head(b)
                    if 1 <= b <= Bshard:
                        emit_score(b - 1, sts[b - 1])
                    if 2 <= b <= Bshard + 1:
                        emit_sel(b - 2, sts[b - 2])
                    if 3 <= b <= Bshard + 2:
                        emit_out(b - 3, sts[b - 3])
                    if b >= 4:
                        fb = b - 4
                        emit_fin(fb, sts.pop(fb))
                        if fb % G == G - 1:
                            chunks.pop(fb // G, None)

    nc.compile()
    return nc


def _get_program(Bshard: int):
    key = ("prog", Bshard)
    if key not in _BASS_STATE:
        _BASS_STATE[key] = _build_program(Bshard)
    return _BASS_STATE[key]


def pack_inputs(hidden: np.ndarray, adj: np.ndarray, apack: np.ndarray):
    """Host-side packing of full inputs into the kernel's DRAM tensors.

    hidden: [B, N, D] f32; adj: [B, N, N] int; apack: [4, D] f32.
    Returns dict of full (unsharded) arrays: in8, in16 (chunk-shaped per core
    after the caller splits the leading dim).
    """
    import ml_dtypes

    F8 = ml_dtypes.float8_e4m3
    B, N, D = hidden.shape
    P = 128
    f32 = np.float32

    hT = np.ascontiguousarray(hidden.transpose(0, 2, 1)).astype(f32)  # [B, D, N]
    hThi = hT.astype(F8)
    hTlo = (hT - hThi.astype(f32)).astype(F8)

    wht = np.zeros((B, P, 8, 128), dtype=F8)
    wht[:, :, 0, :] = hTlo[:, :, 0:128]
    wht[:, :, 1, :] = hTlo[:, :, 128:256]
    wht[:, :, 2, :] = hThi[:, :, 0:128]
    wht[:, :, 3, :] = hThi[:, :, 128:256]
    wht[:, :, 4, :] = wht[:, :, 2, :]
    wht[:, :, 5, :] = wht[:, :, 3, :]
    ident = np.eye(128, dtype=f32)
    wht[:, :, 6, :] = (BIG * ident).astype(F8)[None]
    wht[:, :, 7, :] = (-BIG * ident).astype(F8)[None]

    # hw_r = a_r (.) hT, hi/lo fp8 split, relation plane order [r0, r1, r3, r2]
    hwf = apack[None, :, :, None] * hT[:, None, :, :]  # [B, 4, D, N]
    hwhi = hwf.astype(F8)
    hwlo = (hwf - hwhi.astype(f32)).astype(F8)
    mv = np.zeros((B, P, 6, 512), dtype=F8)
    mv[:, :, 0, 0:256] = hwhi[:, 0]
    mv[:, :, 0, 256:512] = hwhi[:, 1]
    mv[:, :, 1, 0:256] = hwhi[:, 3]
    mv[:, :, 1, 256:512] = hwhi[:, 2]
    mv[:, :, 2, 0:256] = hwlo[:, 0]
    mv[:, :, 2, 256:512] = hwlo[:, 1]
    mv[:, :, 3, 0:256] = hwlo[:, 3]
    mv[:, :, 3, 256:512] = hwlo[:, 2]
    # masks S = [adj==1]-[adj==4], T = [adj==2]-[adj==3], transposed to [j, i]
    adjT = adj.transpose(0, 2, 1)
    S = ((adjT == 1).astype(f32) - (adjT == 4)).astype(F8)  # [B, j, i]
    T = ((adjT == 2).astype(f32) - (adjT == 3)).astype(F8)
    mv[:, :, 4, 0:256] = S[:, 0:128, :]
    mv[:, :, 4, 256:512] = T[:, 0:128, :]
    mv[:, :, 5, 0:256] = S[:, 128:256, :]
    mv[:, :, 5, 256:512] = T[:, 128:256, :]

    xt = np.ones((B, P, 2, 129), dtype=np.float16)
    h4 = hidden.reshape(B, 2, 128, D)  # [B, I, p, D]
    xt[:, :, :, 0:128] = h4.transpose(0, 2, 1, 3).astype(np.float16)

    # chunk packing: [B, P, bytes] -> [B//G, P, G, bytes]
    in8 = np.concatenate(
        [wht.reshape(B, P, 1024), mv.reshape(B, P, 3072)], axis=2
    )  # [B, P, 4096] f8
    in8 = np.ascontiguousarray(
        in8.reshape(B // G, G, P, 4096).transpose(0, 2, 1, 3)
    )  # [B/G, P, G, 4096]
    in16 = np.ascontiguousarray(
        xt.reshape(B // G, G, P, 258).transpose(0, 2, 1, 3)
    )  # [B/G, P, G, 258] f16
    return {"in8": in8, "in16": in16}


def unpack_output(out: np.ndarray) -> np.ndarray:
    """[B/G, P, G, I, d] f16 -> [B, N, D] f32."""
    nch = out.shape[0]
    B = nch * G
    return (
        out.transpose(0, 2, 3, 1, 4).reshape(B, 256, 128).astype(np.float32)
    )


def kernel(hidden: np.ndarray, adj: np.ndarray, a_0, a_1, a_2, a_3) -> np.ndarray:
    from concourse import bass_utils

    B, N, D = hidden.shape
    NCORES = 8
    assert B % NCORES == 0
    Bs = B // NCORES

    apack = np.ascontiguousarray(
        np.concatenate([a_0, a_1, a_2, a_3], axis=1).T.astype(np.float32)
    )  # [4, D]
    hidden = np.ascontiguousarray(hidden, dtype=np.float32)
    packed = pack_inputs(hidden, np.asarray(adj), apack)

    nc = _get_program(Bs)
    nchunk = Bs // G
    in_maps = [
        {k: v[c * nchunk : (c + 1) * nchunk] for k, v in packed.items()}
        for c in range(NCORES)
    ]
    res = bass_utils.run_bass_kernel_spmd(
        nc,
        in_maps,
        core_ids=list(range(NCORES)),
        trace=bool(int(os.environ.get("KERNEL_TRACE", "0"))),
    )
    _BASS_STATE["last_result"] = res
    return unpack_output(np.concatenate([r["out"] for r in res.results], axis=0))
